# revision 8
# baseline (speedup 1.0000x reference)
"""HardBinaryVote Trainium2 kernel.

out[s] = (sum_m w[m]*votes[m,s] > sum_m w[m]/2)  as int32, votes in {0,1}.

Strategy (8 NeuronCores, sample-sharded):
  - Each core gets a [63, 250000] shard of votes, folded host-side into
    [126, 125000] (two 125k sample half-shards stacked on the partition axis
    so K=126 of the PE's 128 contraction rows are used -> 2 samples/col).
  - SWDGE DMA casts int32 -> fp16 in flight (votes are 0/1, exact in fp16).
  - Weights are split w = hi + lo (fp16 each) and laid out as [126, 2] lhsT
    columns; two accumulating matmuls per sub-chunk give fp32-accuracy
    weighted sums c1 in PSUM [2, 500].
  - DVE tensor_scalar(is_gt, T=sum(w)/2) thresholds PSUM -> int32 SBUF,
    batched 4 PSUM banks per op; results DMA back per 16 sub-chunks.
"""

import sys

import numpy as np

sys.path.insert(0, "/opt/trn_rl_repo")

from concourse import bacc, bass_utils, mybir, tile  # noqa: E402

N_MODELS = 63
N_SAMPLES = 2_000_000
N_CORES = 8
S_CORE = N_SAMPLES // N_CORES  # 250000 samples per core
H = S_CORE // 2  # 125000 group-columns per core
KP = 2 * N_MODELS  # 126 contraction rows

C_SUB = 500  # matmul free dim (one PSUM bank holds 512 fp32)
N_SUB = H // C_SUB  # 250 sub-chunks per core
DMA_SUB = 10  # sub-chunks per input DMA tile (5000 cols)
PSUM_SUB = 4  # sub-chunks per PSUM tile (4 banks)
OUT_SUB = 16  # sub-chunks per output tile

_last_results = None  # BassKernelResults of the most recent run (for test.py)


def _build_program(threshold: float):
    nc = bacc.Bacc("TRN2", target_bir_lowering=False, debug=False)

    votes_d = nc.dram_tensor("votes", [KP, H], mybir.dt.int32, kind="ExternalInput")
    whi_d = nc.dram_tensor("whi", [KP, 2], mybir.dt.float16, kind="ExternalInput")
    wlo_d = nc.dram_tensor("wlo", [KP, 2], mybir.dt.float16, kind="ExternalInput")
    out_d = nc.dram_tensor("out", [2, H], mybir.dt.int32, kind="ExternalOutput")

    with tile.TileContext(nc) as tc:
        with (
            tc.tile_pool(name="w", bufs=1) as wpool,
            tc.tile_pool(name="v", bufs=8) as vpool,
            tc.tile_pool(name="m", bufs=4) as mpool,
            tc.tile_pool(name="o", bufs=2) as opool,
            tc.tile_pool(name="ps", bufs=2, space="PSUM") as ppool,
        ):
            whi_sb = wpool.tile([KP, 2], mybir.dt.float16, tag="whi")
            wlo_sb = wpool.tile([KP, 2], mybir.dt.float16, tag="wlo")
            nc.sync.dma_start(out=whi_sb[:], in_=whi_d[:])
            nc.sync.dma_start(out=wlo_sb[:], in_=wlo_d[:])

            vt = None
            ps = None
            ot = None
            o_base = 0
            for j in range(N_SUB):
                d, dj = divmod(j, DMA_SUB)
                if dj == 0:
                    vt = vpool.tile([KP, DMA_SUB * C_SUB], mybir.dt.float16)
                    nc.gpsimd.dma_start(
                        out=vt[:],
                        in_=votes_d[:, d * DMA_SUB * C_SUB : (d + 1) * DMA_SUB * C_SUB],
                    )
                g_off = j % PSUM_SUB
                if g_off == 0:
                    ps = ppool.tile([2, PSUM_SUB, 512], mybir.dt.float32)
                o, oj = divmod(j, OUT_SUB)
                if oj == 0:
                    ot = opool.tile([2, OUT_SUB, 512], mybir.dt.float16)
                    o_base = o * OUT_SUB
                    n_in_otile = min(OUT_SUB, N_SUB - o_base)

                rhs = vt[:, dj * C_SUB : (dj + 1) * C_SUB]
                acc = ps[:, g_off, :C_SUB]
                nc.tensor.matmul(acc, whi_sb[:], rhs, start=True, stop=False)
                nc.tensor.matmul(acc, wlo_sb[:], rhs, start=False, stop=True)

                if g_off == PSUM_SUB - 1 or j == N_SUB - 1:
                    nblk = g_off + 1
                    g_idx = j // PSUM_SUB
                    if g_idx % 3 != 2:
                        # ACT: evacuate PSUM as fp16 margins (c1-T); sign-exact.
                        mt = mpool.tile([2, PSUM_SUB, 512], mybir.dt.float16)
                        nc.scalar.activation(
                            out=mt[:, :nblk, :],
                            in_=ps[:, :nblk, :],
                            func=mybir.ActivationFunctionType.Copy,
                            bias=-float(threshold),
                            scale=1.0,
                        )
                        # DVE: margin > 0 -> 1.0/0.0 fp16 (4x mode, SBUF)
                        nc.vector.tensor_scalar(
                            out=ot[:, oj - nblk + 1 : oj + 1, :],
                            in0=mt[:, :nblk, :],
                            scalar1=0.0,
                            scalar2=None,
                            op0=mybir.AluOpType.is_gt,
                        )
                    else:
                        # DVE: direct threshold from PSUM (1x mode)
                        nc.vector.tensor_scalar(
                            out=ot[:, oj - nblk + 1 : oj + 1, :],
                            in0=ps[:, :nblk, :],
                            scalar1=float(threshold),
                            scalar2=None,
                            op0=mybir.AluOpType.is_gt,
                        )
                if j == o_base + n_in_otile - 1:
                    # SWDGE out-DMA casts fp16 1.0/0.0 -> int32 1/0
                    nc.gpsimd.dma_start(
                        out=out_d[
                            :, o_base * C_SUB : (o_base + n_in_otile) * C_SUB
                        ],
                        in_=ot[:, :n_in_otile, :C_SUB],
                    )

    nc.compile()
    return nc


def kernel(votes: np.ndarray, vote_weights: np.ndarray) -> np.ndarray:
    global _last_results
    votes = np.ascontiguousarray(votes, dtype=np.int32)
    w = np.asarray(vote_weights, dtype=np.float32)
    assert votes.shape == (N_MODELS, N_SAMPLES)

    threshold = float(np.float32(w.astype(np.float64).sum() / 2.0))
    w_hi = w.astype(np.float16)
    w_lo = (w - w_hi.astype(np.float32)).astype(np.float16)
    whi = np.zeros((KP, 2), np.float16)
    wlo = np.zeros((KP, 2), np.float16)
    whi[:N_MODELS, 0] = w_hi
    whi[N_MODELS:, 1] = w_hi
    wlo[:N_MODELS, 0] = w_lo
    wlo[N_MODELS:, 1] = w_lo

    in_maps = []
    for c in range(N_CORES):
        sh = votes[:, c * S_CORE : (c + 1) * S_CORE]
        folded = np.ascontiguousarray(
            np.concatenate([sh[:, :H], sh[:, H:]], axis=0)
        )
        in_maps.append({"votes": folded, "whi": whi, "wlo": wlo})

    nc = _build_program(threshold)
    res = bass_utils.run_bass_kernel_spmd(nc, in_maps, core_ids=list(range(N_CORES)))
    _last_results = res

    out = np.concatenate(
        [res.results[c]["out"].reshape(-1) for c in range(N_CORES)]
    )
    return np.ascontiguousarray(out.astype(np.int32))


# revision 10
# speedup vs baseline: 1.1755x; 1.1755x over previous
"""HardBinaryVote Trainium2 kernel.

out[s] = (sum_m w[m]*votes[m,s] > sum_m w[m]/2)  as int32, votes in {0,1}.

Strategy (8 NeuronCores, sample-sharded):
  - Each core gets a [63, 250000] shard of votes, folded host-side into
    [126, 125000] (two 125k sample half-shards stacked on the partition axis
    so K=126 of the PE's 128 contraction rows are used -> 2 samples/col).
  - SWDGE DMA casts int32 -> fp16 in flight (votes are 0/1, exact in fp16).
  - Weights are split w = hi + lo (fp16 each) and laid out as [126, 2] lhsT
    columns; two accumulating matmuls per sub-chunk give fp32-accuracy
    weighted sums c1 in PSUM [2, 500].
  - DVE tensor_scalar(is_gt, T=sum(w)/2) thresholds PSUM -> int32 SBUF,
    batched 4 PSUM banks per op; results DMA back per 16 sub-chunks.
"""

import sys

import numpy as np

sys.path.insert(0, "/opt/trn_rl_repo")

from concourse import bacc, bass_utils, mybir, tile  # noqa: E402

N_MODELS = 63
N_SAMPLES = 2_000_000
N_CORES = 8
S_CORE = N_SAMPLES // N_CORES  # 250000 samples per core
H = S_CORE // 2  # 125000 group-columns per core
KP = 2 * N_MODELS  # 126 contraction rows

C_SUB = 500  # matmul free dim (one PSUM bank holds 512 fp32)
N_SUB = H // C_SUB  # 250 sub-chunks per core
DMA_SUB = 10  # sub-chunks per input DMA tile (5000 cols)
PSUM_SUB = 4  # sub-chunks per PSUM tile (4 banks)
OUT_SUB = 16  # sub-chunks per output tile

_last_results = None  # BassKernelResults of the most recent run (for test.py)


def _build_program(threshold: float):
    nc = bacc.Bacc("TRN2", target_bir_lowering=False, debug=False)

    votes_d = nc.dram_tensor("votes", [KP, H], mybir.dt.int32, kind="ExternalInput")
    whi_d = nc.dram_tensor("whi", [KP, 2], mybir.dt.float16, kind="ExternalInput")
    wlo_d = nc.dram_tensor("wlo", [KP, 2], mybir.dt.float16, kind="ExternalInput")
    out_d = nc.dram_tensor("out", [2, H], mybir.dt.int32, kind="ExternalOutput")

    with tile.TileContext(nc) as tc:
        with (
            tc.tile_pool(name="w", bufs=1) as wpool,
            tc.tile_pool(name="v", bufs=8) as vpool,
            tc.tile_pool(name="m", bufs=4) as mpool,
            tc.tile_pool(name="o", bufs=2) as opool,
            tc.tile_pool(name="ps", bufs=2, space="PSUM") as ppool,
        ):
            whi_sb = wpool.tile([KP, 2], mybir.dt.float16, tag="whi")
            wlo_sb = wpool.tile([KP, 2], mybir.dt.float16, tag="wlo")
            nc.sync.dma_start(out=whi_sb[:], in_=whi_d[:])
            nc.sync.dma_start(out=wlo_sb[:], in_=wlo_d[:])

            vt = None
            ps = None
            ot = None
            o_base = 0
            for j in range(N_SUB):
                d, dj = divmod(j, DMA_SUB)
                if dj == 0:
                    vt = vpool.tile([KP, DMA_SUB * C_SUB], mybir.dt.float16)
                    nc.gpsimd.dma_start(
                        out=vt[:],
                        in_=votes_d[:, d * DMA_SUB * C_SUB : (d + 1) * DMA_SUB * C_SUB],
                    )
                g_off = j % PSUM_SUB
                if g_off == 0:
                    ps = ppool.tile([2, PSUM_SUB, 512], mybir.dt.float32)
                o, oj = divmod(j, OUT_SUB)
                if oj == 0:
                    ot = opool.tile([2, OUT_SUB, 512], mybir.dt.int32)
                    o_base = o * OUT_SUB
                    n_in_otile = min(OUT_SUB, N_SUB - o_base)

                rhs = vt[:, dj * C_SUB : (dj + 1) * C_SUB]
                acc = ps[:, g_off, :C_SUB]
                nc.tensor.matmul(acc, whi_sb[:], rhs, start=True, stop=False)
                nc.tensor.matmul(acc, wlo_sb[:], rhs, start=False, stop=True)

                if g_off == PSUM_SUB - 1 or j == N_SUB - 1:
                    nblk = g_off + 1
                    g_idx = j // PSUM_SUB
                    if g_idx % 3 != 2:
                        # ACT: evacuate PSUM as fp16 margins (c1-T); sign-exact.
                        mt = mpool.tile([2, PSUM_SUB, 512], mybir.dt.float16)
                        nc.scalar.activation(
                            out=mt[:, :nblk, :],
                            in_=ps[:, :nblk, :],
                            func=mybir.ActivationFunctionType.Copy,
                            bias=-float(threshold),
                            scale=1.0,
                        )
                        # DVE: margin > 0 -> 1.0/0.0 fp16 (4x mode, SBUF)
                        nc.vector.tensor_scalar(
                            out=ot[:, oj - nblk + 1 : oj + 1, :],
                            in0=mt[:, :nblk, :],
                            scalar1=0.0,
                            scalar2=None,
                            op0=mybir.AluOpType.is_gt,
                        )
                    else:
                        # DVE: direct threshold from PSUM (1x mode)
                        nc.vector.tensor_scalar(
                            out=ot[:, oj - nblk + 1 : oj + 1, :],
                            in0=ps[:, :nblk, :],
                            scalar1=float(threshold),
                            scalar2=None,
                            op0=mybir.AluOpType.is_gt,
                        )
                if j == o_base + n_in_otile - 1:
                    nc.sync.dma_start(
                        out=out_d[
                            :, o_base * C_SUB : (o_base + n_in_otile) * C_SUB
                        ],
                        in_=ot[:, :n_in_otile, :C_SUB],
                    )

    nc.compile()
    return nc


def kernel(votes: np.ndarray, vote_weights: np.ndarray) -> np.ndarray:
    global _last_results
    votes = np.ascontiguousarray(votes, dtype=np.int32)
    w = np.asarray(vote_weights, dtype=np.float32)
    assert votes.shape == (N_MODELS, N_SAMPLES)

    threshold = float(np.float32(w.astype(np.float64).sum() / 2.0))
    w_hi = w.astype(np.float16)
    w_lo = (w - w_hi.astype(np.float32)).astype(np.float16)
    whi = np.zeros((KP, 2), np.float16)
    wlo = np.zeros((KP, 2), np.float16)
    whi[:N_MODELS, 0] = w_hi
    whi[N_MODELS:, 1] = w_hi
    wlo[:N_MODELS, 0] = w_lo
    wlo[N_MODELS:, 1] = w_lo

    in_maps = []
    for c in range(N_CORES):
        sh = votes[:, c * S_CORE : (c + 1) * S_CORE]
        folded = np.ascontiguousarray(
            np.concatenate([sh[:, :H], sh[:, H:]], axis=0)
        )
        in_maps.append({"votes": folded, "whi": whi, "wlo": wlo})

    nc = _build_program(threshold)
    res = bass_utils.run_bass_kernel_spmd(nc, in_maps, core_ids=list(range(N_CORES)))
    _last_results = res

    out = np.concatenate(
        [res.results[c]["out"].reshape(-1) for c in range(N_CORES)]
    )
    return np.ascontiguousarray(out.astype(np.int32))
